# revision 26
# baseline (speedup 1.0000x reference)
"""ACE loss kernel for TRN2, data-parallel over 8 NeuronCores.

Math (per sample b, with targets y[b, 0:8] and logits x[b, c, t]):
  m[b,t]   = max_c x[b,c,t]
  cnt[b,j] = #{t : x[b, y[b,j], t] == m[b,t]}        == n_k[b, y[b,j]] (no ties)
  dup[b,j] = multiplicity of y[b,j] within y[b,:]    == y_k[b, y[b,j]]
Only target classes contribute to the masked loss, so the full 128-bin
argmax histogram is never materialized:
  n_sum[b] = sum_j cnt/dup   (each distinct class counted once)
  n_p[b,j] = max(cnt / max(n_sum,1), EPS)
  loss[b]  = sum_j n_p * (-log(dup/8)) / dup
  out      = mean_b loss

Final design (f16 stream, DVE tree + ACT exp-accum counts), measured
68.9us on HW vs the 120.4us f32 baseline.  Per-core budget: 16.8 MiB
f16 stream at ~430 GB/s (~42.5us) vs ~49us of Vector-engine work, so
the DVE is the bottleneck and everything else is scheduled around
keeping it saturated from first byte to last.
  - Host casts x to f16: by monotonicity of round-to-nearest,
    max(f16 a, f16 b) == f16(max(a,b)), so this is bit-identical to the
    old f32 kernel's first-level rounding while HALVING the HBM stream
    (32 -> 16.8 MiB/core).  Measured stream rate ~430 GB/s/core.
  - All tree levels are f16 TT-max at DVE 2x_1P (2 out/cycle; the only
    accelerated DVE op family that can reduce -- tensor_reduce/pool are
    1x, gpsimd TT is rejected by codegen, PE has no max, DMA-CCE max is
    verifier-rejected).  Measured DVE busy matches the cycle model <1%.
  - Scheduling facts measured on HW: (1) a compute op reading a pool
    tile waits for ALL of that tile's DMA writers, so each streamed
    chunk must live in its own tile to pipeline; (2) batched multi-tile
    trees (pair/quad) stall the stream for the whole group -> per-tile
    trees everywhere; (3) runtime preamble holds the sync engine until
    ~7.3us, each dma_start costs ~0.6us of serialized descriptor
    generation, and each DMA completion pays a ~2us receipt before its
    semaphore fires -- so the DVE cannot start before ~12us and
    fine-grained ramp chunking exposes one receipt gap per chunk
    (quarter-chunked ramps measured 3-5us SLOWER overall).
  - Tile 0 loads as two separate HALF tiles (first l1 ~2.8us earlier;
    the second half's receipt hides under the first l1); tiles 1-7
    whole (one DMA each, l1 fires on landing; per-tile DVE work ~5.5us
    > ~4.8us DMA window keeps the engine saturated and ~14us
    backlogged by the last byte, which also makes tail-chunking of
    tile 7 pointless).
  - yg (pre-gathered target rows) rides the ring AFTER tile 2: placed
    between tiles 0 and 1 it delayed tile 1 by 2.5us (a measured 2.9us
    DVE hole); count emissions trail it so the in-order DVE queue
    never blocks on its landing.  yw (epilogue-only metadata) rides
    behind the last x tile.
  - Counts: DVE computes d = m - xg (exact f16 sub, pairs of tiles),
    the otherwise-idle ACT engine accumulates exp(-16384*d) per (k,j)
    row via accum_out (matches contribute exp(0)=1, non-matches
    (d >= ~half-ulp) < 3e-4).  Tile 7 counts on DVE (is_equal+reduce)
    to avoid queueing behind ACT at the tail.
  - dup-derived per-target scalars rd = 1/dup and wgt = -log(dup/8)/dup
    are target-metadata prepared host-side with the gather (y-only).
  - Split epilogue: tiles 0-6's nd/nsum partials run mid-backlog
    (their ACT counts land ~10us before the last tree), so after tile
    7's count only the [p,8]-sized remainder chains into the PE f32
    dot with ones; the scalar leaves through a 4-byte single-packet
    DMA.
Each core returns one f32; the host sums 8 of them and divides by B.
"""

import numpy as np

B, C, T, L = 8192, 128, 64, 8
N_CORES = 8
B_SH = B // N_CORES          # 1024 samples per core
NT = B_SH // 128             # 8 tiles of 128 samples
EPS = 1e-5

_CACHE = {}


def _build_nc():
    import sys
    if "/opt/trn_rl_repo" not in sys.path:
        sys.path.insert(0, "/opt/trn_rl_repo")
    from concourse import bacc, mybir
    from concourse.tile import TileContext

    f32 = mybir.dt.float32
    f16 = mybir.dt.float16
    AX = mybir.AxisListType
    OP = mybir.AluOpType

    CT = C * T            # 8192 elems per sample
    Q = CT // 4           # 2048
    H = CT // 2           # 4096

    nc = bacc.Bacc("TRN2")
    x = nc.declare_dram_parameter("x", [B_SH, CT], f16, isOutput=False)
    XGO = NT * L          # xg columns start here inside yga
    yg = nc.declare_dram_parameter(
        "yg", [128, XGO + NT * L * T], f16, isOutput=False
    )
    # host-precomputed target metadata: [rd | wgt], each [128, NT*L]
    yw = nc.declare_dram_parameter("yw", [128, 2 * NT * L], f32, isOutput=False)
    out = nc.declare_dram_parameter("out", [1, 1], f32, isOutput=True)

    with TileContext(nc) as tc:
        with (
            tc.tile_pool(name="xp", bufs=2) as xp,
            tc.tile_pool(name="hp", bufs=2) as hp,
            tc.tile_pool(name="sp", bufs=3) as sp,
            tc.tile_pool(name="cp", bufs=1) as cp,
            tc.tile_pool(name="ps", bufs=1, space="PSUM") as pp,
        ):
            # ---- whole-run tiles ----
            yga = cp.tile([128, XGO + NT * L * T], f16)
            ywa = cp.tile([128, 2 * NT * L], f32)
            mh = cp.tile([128, NT * T], f16)      # per-tile class-max rows
            cnta = cp.tile([128, NT * L], f32)
            ones = cp.tile([128, 1], f32)
            nc.gpsimd.memset(ones[:, :], 1.0)
            rd = ywa[:, 0:NT * L]
            wgt = ywa[:, NT * L:2 * NT * L]

            # f16 pair-max: xt[lo:lo+n] -> ht[hoff:hoff+n/2]; pairs
            # (lo+i, lo+n/2+i) are 64-aligned so t-columns stay aligned
            def l1(xt, lo, n, ht, hoff):
                nc.vector.tensor_tensor(
                    out=ht[:, hoff:hoff + n // 2],
                    in0=xt[:, lo:lo + n // 2],
                    in1=xt[:, lo + n // 2:lo + n],
                    op=OP.max,
                )

            # fp16 in-place max tree over t[:, lo:lo+n] down to W wide
            def tree16(t, lo, n, W=T, last_out=None):
                w = n
                while w > W:
                    h = w // 2
                    dst = (last_out if (last_out is not None and h == W)
                           else t[:, lo:lo + h])
                    nc.vector.tensor_tensor(
                        out=dst, in0=t[:, lo:lo + h],
                        in1=t[:, lo + h:lo + w], op=OP.max,
                    )
                    w = h

            # DVE computes d = m - xg for tiles [k0, k0+g); ACT turns
            # each (k,j) row into a count via exp(-16384*d) + accum_out
            def count_act(k0, g):
                d = sp.tile([128, 2 * L * T], f16, tag="d")
                e = sp.tile([128, 2 * L * T], f16, tag="e")
                nc.vector.tensor_tensor(
                    out=d[:, 0:g * L * T].rearrange(
                        "p (g l t) -> p g l t", g=g, l=L),
                    in0=mh[:, k0 * T:(k0 + g) * T].rearrange(
                        "p (g t) -> p g t", g=g
                    ).unsqueeze(2).to_broadcast([128, g, L, T]),
                    in1=yga[:, XGO + k0 * L * T:XGO + (k0 + g) * L * T].rearrange(
                        "p (g l t) -> p g l t", g=g, l=L
                    ),
                    op=OP.subtract,
                )
                for s in range(g * L):
                    slot = k0 * L + s
                    nc.scalar.activation(
                        out=e[:, s * T:(s + 1) * T],
                        in_=d[:, s * T:(s + 1) * T],
                        func=mybir.ActivationFunctionType.Exp,
                        scale=-16384.0,
                        accum_out=cnta[:, slot:slot + 1],
                    )

            # DVE-only count (tile 7: no ACT queue at the tail)
            def count_dve(k):
                eq = sp.tile([128, 2 * L * T], f16, tag="d")
                nc.vector.tensor_tensor(
                    out=eq[:, 0:L * T].rearrange("p (l t) -> p l t", l=L),
                    in0=yga[:, XGO + k * L * T:XGO + (k + 1) * L * T].rearrange(
                        "p (l t) -> p l t", l=L
                    ),
                    in1=mh[:, k * T:(k + 1) * T].unsqueeze(1)
                    .to_broadcast([128, L, T]),
                    op=OP.is_equal,
                )
                nc.vector.reduce_sum(
                    out=cnta[:, k * L:(k + 1) * L],
                    in_=eq[:, 0:L * T].rearrange("p (l t) -> p l t", l=L),
                    axis=AX.X,
                )

            # ---- tile 0: two separate HALF tiles.  The first l1 fires
            # ~2.8us earlier than a whole-tile load; the second half's
            # ~2us completion receipt hides under the first half's l1,
            # unlike smaller chunkings which expose every receipt
            # during the ramp (measured: v10's [Q|3Q] split lost 3.5us)
            ht = hp.tile([128, H], f16, tag="hts")
            for c in range(2):
                x0 = xp.tile([128, H], f16, tag="x7", bufs=2)
                nc.sync.dma_start(
                    out=x0[:, :], in_=x[0:128, c * H:(c + 1) * H]
                )
                l1(x0, 0, H, ht, c * Q)
            tree16(ht, 0, H, T, last_out=mh[:, 0:T])

            # ---- tiles 1-7: whole-tile loads, per-tile trees.  One
            # tile = one DMA writer, so l1 fires the moment it lands;
            # per-tile DVE work (~5.3us) > tile DMA window (~4.8us), so
            # the engine stays saturated and is deeply backlogged by
            # the stream end (tail chunking of tile 7 buys nothing).
            for k in range(1, 8):
                ht = hp.tile([128, H], f16, tag="hts")
                row = slice(k * 128, (k + 1) * 128)
                xh = xp.tile([128, CT], f16, tag="xh", bufs=4)
                nc.sync.dma_start(out=xh[:, :], in_=x[row, :])
                if k == 2:
                    # yg rides the ring AFTER tile 2: putting it between
                    # tiles 0 and 1 delayed tile 1 by 2.5us (measured
                    # 2.9us DVE hole); the first count only needs it at
                    # ~27us.  Count emissions below trail it so the
                    # in-order DVE queue never blocks on yg's landing.
                    nc.sync.dma_start(out=yga[:, :], in_=yg[:, :])
                if k == 7:
                    # yw (epilogue-only metadata, 64KB) rides BEHIND the
                    # last x bytes: its DGE slot never delays the stream
                    nc.sync.dma_start(out=ywa[:, :], in_=yw[:, :])
                    count_act(6, 1)       # tree(6) done; yga long landed
                l1(xh, 0, CT, ht, 0)
                tree16(ht, 0, H, T, last_out=mh[:, k * T:(k + 1) * T])
                if k in (2, 4, 6):
                    count_act(k - 2, 2)   # pairs (0,1), (2,3), (4,5)

            # ---- batched epilogue over all 8 tile columns ----
            nd = cp.tile([128, NT * L], f32)
            nsum = cp.tile([128, NT], f32)
            inv = cp.tile([128, NT], f32)
            npj = cp.tile([128, NT * L], f32)
            lj = cp.tile([128, NT * L], f32)
            # tiles 0-6's nd/nsum only need the ACT counts (done mid-
            # backlog); emitted before count_dve(7) they shorten the
            # serial tail after the last count
            S7 = 7 * L
            nc.vector.tensor_mul(
                out=nd[:, 0:S7], in0=cnta[:, 0:S7], in1=rd[:, 0:S7])
            nc.vector.reduce_sum(
                out=nsum[:, 0:7],
                in_=nd[:, 0:S7].rearrange("p (k j) -> p k j", j=L),
                axis=AX.X,
            )
            count_dve(7)
            nc.vector.tensor_mul(
                out=nd[:, S7:], in0=cnta[:, S7:], in1=rd[:, S7:])
            nc.vector.reduce_sum(
                out=nsum[:, 7:8],
                in_=nd[:, S7:].rearrange("p (k j) -> p k j", j=L),
                axis=AX.X,
            )
            nc.vector.tensor_scalar_max(out=nsum[:, :], in0=nsum[:, :], scalar1=1.0)
            nc.vector.reciprocal(out=inv[:, :], in_=nsum[:, :])
            nc.vector.tensor_tensor(
                out=npj[:, :].rearrange("p (k j) -> p k j", j=L),
                in0=cnta[:, :].rearrange("p (k j) -> p k j", j=L),
                in1=inv[:, :].unsqueeze(2).to_broadcast([128, NT, L]),
                op=OP.mult,
            )
            # lj = max(npj, EPS) * wgt in one STT op
            nc.vector.scalar_tensor_tensor(
                out=lj[:, :], in0=npj[:, :], scalar=EPS,
                in1=wgt, op0=OP.max, op1=OP.mult,
            )
            acc = cp.tile([128, 1], f32)
            nc.vector.reduce_sum(
                out=acc[:, :],
                in_=lj[:, :].rearrange("p (k j) -> p k j", j=L),
                axis=AX.XY,
            )
            # collapse partitions: PE f32 dot with ones -> PSUM [1,1] -> SBUF
            psc = pp.tile([1, 1], f32)
            nc.tensor.matmul(psc[:, :], acc[:, :], ones[:, :],
                             start=True, stop=True)
            outv = cp.tile([1, 1], f32)
            nc.vector.tensor_copy(out=outv[:, :], in_=psc[:, :])
            nc.sync.dma_start(out=out[:, :], in_=outv[:, :], single_packet=True)
    nc.compile()
    return nc


def _shard_inputs(x, y, target_lengths):
    """Numpy-side sharding, f16 cast, target-row pre-gather, layouts."""
    x = np.asarray(x, dtype=np.float32)
    y = np.asarray(y, dtype=np.int32)
    y2 = y.reshape(B, L)  # target_lengths is L for every sample (spec'd)
    x3 = x.reshape(B, C, T)
    # f16 stream: max(f16 a, f16 b) == f16(max(a, b)) (monotone rounding),
    # so device results are identical to computing f16 maxes on f32 input
    x16 = x3.astype(np.float16)
    xg_all = np.take_along_axis(
        x16, y2[:, :, None].astype(np.int64), axis=1
    )
    # target metadata: dup multiplicity -> rd = 1/dup, wgt = -log(dup/8)/dup
    dup_all = (y2[:, :, None] == y2[:, None, :]).sum(-1).astype(np.float32)

    def tile_layout(a):  # [B_SH, L] -> [128, NT*L] in (k j) column order
        return np.ascontiguousarray(
            a.reshape(NT, 128, L).transpose(1, 0, 2).reshape(128, -1)
        )

    in_maps = []
    for i in range(N_CORES):
        sl = slice(i * B_SH, (i + 1) * B_SH)
        xs = np.ascontiguousarray(x16[sl].reshape(B_SH, C * T))
        xgs = xg_all[sl].reshape(NT, 128, L * T).transpose(1, 0, 2).reshape(128, -1)
        # classes 0..127 are exact in fp16; pack [yc | xg] as one buffer
        ycs = (y2[sl].reshape(NT, 128, L).transpose(1, 0, 2)
               .reshape(128, -1).astype(np.float16))
        ygs = np.ascontiguousarray(np.concatenate([ycs, xgs], axis=1))
        dup = dup_all[sl]
        rd = tile_layout(1.0 / dup)
        wgt = tile_layout(-np.log(dup / L) / dup)
        yws = np.ascontiguousarray(
            np.concatenate([rd, wgt], axis=1).astype(np.float32)
        )
        in_maps.append({"x": xs, "yg": ygs, "yw": yws})
    return in_maps


def kernel(x, y, target_lengths):
    import sys
    if "/opt/trn_rl_repo" not in sys.path:
        sys.path.insert(0, "/opt/trn_rl_repo")
    from concourse.bass_utils import run_bass_kernel_spmd

    if "nc" not in _CACHE:
        _CACHE["nc"] = _build_nc()
    nc = _CACHE["nc"]

    in_maps = _shard_inputs(x, y, target_lengths)
    res = run_bass_kernel_spmd(nc, in_maps, core_ids=list(range(N_CORES)))
    total = np.float64(0.0)
    for r in res.results:
        total += np.float64(np.asarray(r["out"]).reshape(()))
    return np.float32(total / B)
